# revision 26
# baseline (speedup 1.0000x reference)
"""Trainium2 Bass kernel for nn_AmplifierAttention (sparse sigmoid-threshold attention).

  t    = text @ W^T + b          [S, E]
  attn = t @ L^T                 [S, Lb]
  a    = sigmoid(attn); a[a < 0.4] = 0
  out  = softmax(a, axis=-1) @ L [S, E]

Strategy: data-parallel over batch B=8 -> one batch element per NeuronCore,
weights replicated, zero collectives.  Per core, everything is computed
transposed (contraction dims on partitions) so no on-chip transposes are
needed; the host pre-transposes text/W/L instead.

Softmax trick: softmax(a) @ L == (exp(a - c) @ L) / rowsum(exp(a - c)) for any
constant c.  With h = tanh(attn/2) (same ACT table set as exp), sigmoid =
(h+1)/2 and the thresholded exponent is exp(0.5*u - 0.5) where
u = (h+1)*[h >= -0.2]  (sigmoid(x) < 0.4  <=>  tanh(x/2) < -0.2).
The rowsum comes free from a ones-column appended to L in the last matmul.

Precision: fp16 matmul inputs (fp32 PSUM accumulation) run at the same PE
rate as bf16 but with 8x finer mantissa -> base rel err ~2.3e-3 (vs 7.4e-3
bf16).  The freed error budget buys fp8e4 DoubleRow (2 MACs/cell, ~2x) for
the last 2*NP8 l-tiles of the output matmul.  The fp8 stationary stores
v = w - 1 instead of w (|v| ~ 0.4 vs |w| ~ 1.1 -> 2.5x less quantization
noise); the dropped "+1" contribution, sum over the fp8 labels, is a
constant row vector added back on VectorE during evacuation (the rowsum's
count rides the Reciprocal bias).  Measured rel err 1.83e-2 vs the 2e-2 gate.
"""

import os
import numpy as np
import ml_dtypes

P = 128
F16 = np.float16
F8 = ml_dtypes.float8_e4m3fn

NP8 = 10             # l-PAIRS of step 3 run in fp8 DoubleRow (2 l-tiles each)
EPAD = 784           # fp8 label row pitch: 768 features + ones col, %16 == 0

_PROGRAM_CACHE = {}


def build_program(S=2048, DT=1024, E=768, L=4096, SC=512):
    """Build + compile the per-core Bass program (same SPMD program on all cores)."""
    from contextlib import ExitStack
    import concourse.bass as bass  # noqa: F401
    import concourse.mybir as mybir
    import concourse.tile as tile
    from concourse.tile import add_dep_helper
    from concourse import bacc

    def _raw(i):
        return getattr(i, "ins", i)

    dt = mybir.dt
    AF = mybir.ActivationFunctionType
    OP = mybir.AluOpType
    DR = mybir.MatmulPerfMode.DoubleRow

    ND = DT // P        # d-tiles
    NE = E // P         # e-tiles
    NL = L // P         # l-tiles
    NPAIR = NL // 2     # l-pairs (two l-tiles share one 2-bank PSUM tile)
    NP16 = NPAIR - NP8  # l-pairs of step 3 in fp16
    NL16 = 2 * NP16     # l-tiles of step 3 in fp16
    NCH = S // SC       # s-chunks
    NSS = SC // P       # s-subtiles per chunk
    EH = E // 2         # half of the output feature dim
    EP = E + 2          # padded label row: E features + ones col + zero pad
    SSLOT = 2 * max(SC, 512)   # psum slot width (fp32), 2 banks
    HOFF = SSLOT // 2          # bank-aligned offset of the low-half accumulator

    nc = bacc.Bacc("TRN2", target_bir_lowering=False, debug=False)

    NCH_ = S // SC
    # tt/wt arrive host-packed in SBUF tile order (one contiguous run per
    # partition per transfer) — the startup stream then runs near peak HBM
    # bandwidth instead of the ~200GB/s that 1KB strided runs achieve
    tt = nc.dram_tensor("tt", [P, NCH_, DT // P, SC], dt.float16,
                        kind="ExternalInput").ap()
    wt = nc.dram_tensor("wt", [P, E // P, DT // P, P], dt.float16,
                        kind="ExternalInput").ap()
    lt = nc.dram_tensor("lt", [E, L], dt.float16, kind="ExternalInput").ap()
    laug = nc.dram_tensor("laug", [NL16 * P, EP], dt.float16,
                          kind="ExternalInput").ap()
    la8 = nc.dram_tensor("la8", [NP8 * 2 * P, EPAD], dt.float8e4,
                         kind="ExternalInput").ap()
    c8 = nc.dram_tensor("c8", [P, EP], dt.float32,
                        kind="ExternalInput").ap()
    bb = nc.dram_tensor("bb", [E], dt.float32, kind="ExternalInput").ap()
    out = nc.dram_tensor("out", [S, E], dt.float32, kind="ExternalOutput").ap()

    with tile.TileContext(nc) as tc, ExitStack() as ctx:
        const_pool = ctx.enter_context(tc.tile_pool(name="const", bufs=1))
        tt_pool = ctx.enter_context(tc.tile_pool(name="ttp", bufs=1))
        t_pool = ctx.enter_context(tc.tile_pool(name="tp", bufs=1))
        w_pool = ctx.enter_context(tc.tile_pool(name="wp", bufs=1))
        ew_pool = ctx.enter_context(tc.tile_pool(name="ewp", bufs=2))
        o_pool = ctx.enter_context(tc.tile_pool(name="op", bufs=1))
        r_pool = ctx.enter_context(tc.tile_pool(name="rp", bufs=2))
        # one shared PSUM pool: 4 slots x 2 banks = all 8 banks.  The three
        # phases are PE-serial, so sharing gives attention 4-deep buffering
        # and step 3 all four accumulator pairs in a single pass.
        pp_pool = ctx.enter_context(tc.tile_pool(name="pp", bufs=4, space="PSUM"))

        # --- resident weights.  The scalar HWDGE ring (a second,
        # independent FIFO) carries ALL the weight pairs; the sync ring's
        # head belongs to the text quarters that pace step-1 group 0, then
        # bias + label pieces.  Both rings' first descriptors complete
        # ~12.5us (latency-bound), so group 0 starts ~1.5us sooner and is
        # never text-supply paced.
        wt_sb = const_pool.tile([P, NE, ND, P], dt.float16, tag="wt")
        EG = min(2, NE)
        for g0 in range(0, 2 * EG, EG):
            g1 = min(g0 + EG, NE)
            nc.scalar.dma_start(wt_sb[:, g0:g1], wt[:, g0:g1])
        tt0_sb = tt_pool.tile([P, ND, SC], dt.float16, tag="tt")
        dstep = max(1, ND // 4)
        for d0 in range(0, ND, dstep):
            d1 = min(d0 + dstep, ND)
            nc.sync.dma_start(tt0_sb[:, d0:d1, :], tt[:, 0, d0:d1, :])
        if NE > 2 * EG:
            # last weight pair on the sync ring, right behind the text: the
            # slow-ramping scalar ring would deliver it ~16us, after group 2
            # could already start; the sync ring lands it ~14.5us
            nc.sync.dma_start(wt_sb[:, 2 * EG:], wt[:, 2 * EG:])
        b_sb = const_pool.tile([P, NE], dt.float32, tag="b")
        nc.sync.dma_start(b_sb[:], bb.rearrange("(a p) -> p a", p=P))
        nbias = const_pool.tile([P, 1], dt.float32, tag="nb")
        nc.vector.memset(nbias[:], -0.5)

        # HAM warm-up: ~5us of throwaway matmuls on a memset tile, ordered
        # ahead of the real stream.  They run in the otherwise-idle window
        # while the first weight/text DMAs are in flight, so the PE clock
        # gate (K=4/8 cold -> 8/8 warm after ~3.4us of sustained activity)
        # flips BEFORE real matmuls start -- the whole kernel runs warm.
        dum = const_pool.tile([P, SC], dt.float16, tag="dum")
        nc.vector.memset(dum[:], 0.0)
        dum_ps = pp_pool.tile([P, SSLOT], dt.float32, tag="pp", name="dummy")
        last_dummy = None
        # 9 dummies = 3.8us cold @2.4GHz (covers the 3.4us HAM window) but
        # still ends before real data (~11.7us) at the 2.0GHz P0 state,
        # where 12 cold dummies would overshoot and delay the real stream
        NDUM = 9
        for i in range(NDUM):
            last_dummy = nc.tensor.matmul(
                dum_ps[:, :SC], lhsT=dum[:, 0:P], rhs=dum[:],
                start=(i == 0), stop=(i == NDUM - 1))

        tt_sbs = {0: tt0_sb}

        # lt streams in ascending-size l-pieces (each piece covers all
        # e-tiles in ONE descriptor) so the first attention pairs wait only
        # for the first ~0.4MB piece; fp16 labels in two halves + the fp8
        # tail tiles, all landing well before step-3 consumes them.
        lt_sb = const_pool.tile([P, NE, L], dt.float16, tag="lt")
        lt_r = lt.rearrange("(a p) l -> p a l", p=P)
        lt_cuts = sorted({0, min(L // 16, L), min(L // 8, L), min(L // 4, L),
                          min(L // 2, L), L})
        for lo, hi in zip(lt_cuts, lt_cuts[1:]):
            nc.sync.dma_start(lt_sb[:, :, lo:hi], lt_r[:, :, lo:hi])
        la_sb = const_pool.tile([P, NL16, EP], dt.float16, tag="la")
        la_r = laug.rearrange("(a p) e -> p a e", p=P)
        for li in range(0, NL16, NL16 // 2):
            nc.sync.dma_start(la_sb[:, li:li + NL16 // 2, :],
                              la_r[:, li:li + NL16 // 2, :])
        la8_sb = const_pool.tile([P, NP8, 2, EPAD], dt.float8e4, tag="la8")
        la8_r = la8.rearrange("(a k p) e -> p a k e", p=P, k=2)
        nc.sync.dma_start(la8_sb[:], la8_r[:])
        # constant-correction operands: sum_fp8(L) replicated on all 128
        # partitions, added on VectorE during evacuation (not on the PE);
        # the rowsum's "+count" correction rides the Reciprocal's bias
        c8_sb = const_pool.tile([P, EP], dt.float32, tag="c8")
        nc.sync.dma_start(c8_sb[:], c8[:])

        def do_step1(cc):
            # step 1: t^T[e, s] = sum_d W^T[d,e] * text^T[d,s]  (+ bias).
            # d-outer over groups of <=4 e-tiles: each arriving text d-tile
            # feeds several matmuls, so chunk-0 is not paced by the text DMA
            t_sb = t_pool.tile([P, NE, SC], dt.float16, tag="t",
                               name=f"t_{cc}")
            # chunk 0's first matmul is ordered behind the HAM warm-up chain
            prev_anchor = last_dummy if cc == 0 else None
            for eg0 in range(0, NE, 2):
                eg = range(eg0, min(eg0 + 2, NE))
                pss = {e: pp_pool.tile([P, SSLOT], dt.float32, tag="pp",
                                       name=f"ps_{cc}_{e}") for e in eg}
                for d in range(ND):
                    for e in eg:
                        m = nc.tensor.matmul(
                            pss[e][:, :SC],
                            lhsT=wt_sb[:, e, d, :],
                            rhs=tt_sbs[cc][:, d, :],
                            start=(d == 0), stop=(d == ND - 1),
                        )
                        if prev_anchor is not None:
                            # stop the scheduler hoisting this group's
                            # slot-waiting matmuls ahead of the work whose
                            # evacuation releases the slots
                            add_dep_helper(_raw(m), _raw(prev_anchor),
                                           sync=False,
                                           reason="step1 group order")
                            prev_anchor = None
                for e in eg:
                    last_evac = nc.scalar.activation(
                        t_sb[:, e, :], pss[e][:, :SC],
                        AF.Identity, bias=b_sb[:, e:e + 1])
                prev_anchor = last_evac
            return t_sb, last_evac

        for c in range(NCH):
            s0 = c * SC
            t_sb, anchor = do_step1(c)
            if c + 1 < NCH:
                # prefetch next chunk's text now so its DMA sits ahead of
                # this chunk's output stores on the sync FIFO; the ordering
                # edge stops the scheduler hoisting it ahead of the loads
                # whose consumers release its slot
                tt_sbs[c + 1] = tt_pool.tile([P, ND, SC], dt.float16,
                                             tag="tt", name=f"tt{c + 1}")
                d = nc.sync.dma_start(tt_sbs[c + 1][:],
                                      tt[:, c + 1])
                add_dep_helper(_raw(d), _raw(anchor), sync=False,
                               reason="tt prefetch after this chunk's step1")

            # ---- step 2: attn^T[l, s] per l-pair + elementwise -> w.
            # Pairs 0..NP16-1 produce fp16 w; pairs NP16.. produce fp8 w8
            # (consumed by the DoubleRow tail of step 3).
            w_sb = w_pool.tile([P, NP16, 2 * SC], dt.float16, tag="w")
            w8_sb = w_pool.tile([P, NP8, 2, SC], dt.float8e4, tag="w8")
            for pr in range(NPAIR):
                pa_full = pp_pool.tile([P, SSLOT], dt.float32, tag="pp")
                pa = pa_full[:, :2 * SC]
                for sub in range(2):
                    li = 2 * pr + sub
                    for e in range(NE):
                        nc.tensor.matmul(
                            pa[:, sub * SC:(sub + 1) * SC],
                            lhsT=lt_sb[:, e, li * P:(li + 1) * P],
                            rhs=t_sb[:, e, :],
                            start=(e == 0), stop=(e == NE - 1),
                        )
                h = ew_pool.tile([P, 2 * SC], dt.float16, tag="h")
                nc.scalar.activation(h[:], pa[:], AF.Tanh, scale=0.5)
                hp1 = ew_pool.tile([P, 2 * SC], dt.float16, tag="hp1")
                nc.vector.tensor_scalar(hp1[:], h[:], 1.0, None, OP.add)
                msk = ew_pool.tile([P, 2 * SC], dt.float16, tag="m")
                nc.vector.tensor_scalar(msk[:], h[:], -0.2, None, OP.is_ge)
                u = ew_pool.tile([P, 2 * SC], dt.float16, tag="u")
                nc.vector.tensor_tensor(u[:], hp1[:], msk[:], OP.mult)
                if pr < NP16:
                    nc.scalar.activation(w_sb[:, pr, :], u[:], AF.Exp,
                                         bias=nbias[:], scale=0.5)
                else:
                    # fp8 pairs store v = w - 1 (see module docstring)
                    e16 = ew_pool.tile([P, 2 * SC], dt.float16, tag="e16")
                    nc.scalar.activation(e16[:], u[:],
                                         AF.Exp, bias=nbias[:], scale=0.5)
                    nc.vector.tensor_scalar(w8_sb[:, pr - NP16, :, :],
                                            e16[:], -1.0, None, OP.add)

            # ---- step 3: out[s, :] = (w @ [L | 1]) / rowsum.  One psum slot
            # per s-subtile holds both e-half accumulators (bank-aligned
            # halves), so all NSS subtiles run in a single pass over l and
            # the two matmuls sharing one stationary w-tile are adjacent.
            # The fp16 l-tiles accumulate first, then the fp8 tail pairs via
            # DoubleRow (one 256-deep matmul per pair+half).  The upper half
            # carries the ones column -> rowsum; its evacuation goes to
            # ScalarE (Copy with per-partition scale) while VectorE handles
            # the lower half, so they overlap.
            def mm16(dst, pr, sub, ss, ecols, first):
                lhsT = w_sb[:, pr, sub * SC + ss * P:sub * SC + (ss + 1) * P]
                return nc.tensor.matmul(
                    dst, lhsT=lhsT, rhs=la_sb[:, 2 * pr + sub, ecols],
                    start=first, stop=False)

            def mm8(dst, pr8, ss, ecols, last):
                lhsT = w8_sb[:, pr8, :, ss * P:(ss + 1) * P]
                return nc.tensor.matmul(
                    dst, lhsT=lhsT, rhs=la8_sb[:, pr8, :, ecols],
                    start=False, stop=last, perf_mode=DR)


            UC = slice(EH, E + 1)   # upper-half feature cols + ones col
            LC = slice(0, EH)       # lower-half feature cols

            out_sb = o_pool.tile([P, NSS, E], dt.float32, tag="osb")
            rinv = r_pool.tile([P, NSS], dt.float32, tag="rinv")
            for ss in range(NSS):
                slot = pp_pool.tile([P, SSLOT], dt.float32, tag="pp",
                                    name=f"po_{c}_{ss}")
                final = (c == NCH - 1 and ss == NSS - 1)
                if final:
                    # kernel tail: run the whole upper-half chain (with the
                    # rowsum column) first, then the lower half.  The lower
                    # half goes to a SEPARATE psum tile: with both halves in
                    # one tile, the upper evacuation's read serializes the
                    # lower chain's writes (~1us PE stall); split tiles give
                    # independent dep tracking, so the upper evac + store
                    # fully overlap the lower chain and only the lower
                    # evac+store trail the last matmul (~2us instead of ~4us).
                    slot_l = pp_pool.tile([P, SSLOT], dt.float32, tag="pp",
                                          name="fin_l")
                    for li in range(NL16):
                        mm16(slot[:, :EH + 1], li // 2, li % 2, ss, UC,
                             li == 0)
                    for pr8 in range(NP8):
                        mm8(slot[:, :EH + 1], pr8, ss, UC, pr8 == NP8 - 1)
                    rt = r_pool.tile([P, 1], dt.float32, tag="rt")
                    nc.vector.tensor_scalar(rt[:], slot[:, EH:EH + 1],
                                            float(NP8 * 2 * P), None,
                                            OP.add)
                    nc.vector.reciprocal(rinv[:, ss:ss + 1], rt[:])
                    tu = ew_pool.tile([P, EH], dt.float32, tag="tu")
                    nc.vector.tensor_tensor(tu[:], slot[:, :EH],
                                            c8_sb[:, EH:E], OP.add)
                    nc.scalar.activation(out_sb[:, ss, EH:E],
                                         tu[:], AF.Copy,
                                         scale=rinv[:, ss:ss + 1])
                    nc.sync.dma_start(
                        out[s0 + ss * P:s0 + (ss + 1) * P, EH:E],
                        out_sb[:, ss, EH:E])
                    for li in range(NL16):
                        mm16(slot_l[:, :EH], li // 2, li % 2, ss, LC,
                             li == 0)
                    for pr8 in range(NP8):
                        mm8(slot_l[:, :EH], pr8, ss, LC, pr8 == NP8 - 1)
                    # two column pieces: the first store issues while the
                    # second piece is still evacuating, shortening the
                    # post-last-matmul chain by ~0.45us
                    EQ = EH // 2
                    for q0 in range(0, EH, EQ):
                        q1 = q0 + EQ
                        tl = ew_pool.tile([P, EQ], dt.float32, tag="tl")
                        nc.vector.tensor_tensor(tl[:], slot_l[:, q0:q1],
                                                c8_sb[:, q0:q1], OP.add)
                        nc.vector.tensor_scalar(out_sb[:, ss, q0:q1],
                                                tl[:],
                                                rinv[:, ss:ss + 1], None,
                                                OP.mult)
                        nc.sync.dma_start(
                            out[s0 + ss * P:s0 + (ss + 1) * P, q0:q1],
                            out_sb[:, ss, q0:q1])
                    continue
                for li in range(NL16):
                    mm16(slot[:, :EH + 1], li // 2, li % 2, ss, UC, li == 0)
                    mm16(slot[:, HOFF:HOFF + EH], li // 2, li % 2, ss, LC,
                         li == 0)
                for pr8 in range(NP8):
                    mm8(slot[:, :EH + 1], pr8, ss, UC, pr8 == NP8 - 1)
                    mm8(slot[:, HOFF:HOFF + EH], pr8, ss, LC,
                        pr8 == NP8 - 1)
                # evacuate this subtile while the next one accumulates:
                # ScalarE takes the upper half, VectorE the lower half
                rt = r_pool.tile([P, 1], dt.float32, tag="rt")
                nc.vector.tensor_scalar(rt[:], slot[:, EH:EH + 1],
                                        float(NP8 * 2 * P), None, OP.add)
                nc.vector.reciprocal(rinv[:, ss:ss + 1], rt[:])
                tu = ew_pool.tile([P, EH], dt.float32, tag="tu")
                nc.vector.tensor_tensor(tu[:], slot[:, :EH],
                                        c8_sb[:, EH:E], OP.add)
                nc.scalar.activation(out_sb[:, ss, EH:E],
                                     tu[:], AF.Copy,
                                     scale=rinv[:, ss:ss + 1])
                nc.sync.dma_start(out[s0 + ss * P:s0 + (ss + 1) * P, EH:E],
                                  out_sb[:, ss, EH:E])
                tl = ew_pool.tile([P, EH], dt.float32, tag="tl")
                nc.vector.tensor_tensor(tl[:], slot[:, HOFF:HOFF + EH],
                                        c8_sb[:, 0:EH], OP.add)
                nc.vector.tensor_scalar(out_sb[:, ss, 0:EH],
                                        tl[:],
                                        rinv[:, ss:ss + 1], None, OP.mult)
                nc.sync.dma_start(out[s0 + ss * P:s0 + (ss + 1) * P, 0:EH],
                                  out_sb[:, ss, 0:EH])

    nc.compile()
    return nc


def _get_program(key):
    if key not in _PROGRAM_CACHE:
        _PROGRAM_CACHE[key] = build_program(*key)
    return _PROGRAM_CACHE[key]


def prep_inputs(text_vec, labels_vec, W_proj, b_proj):
    """Host-side shard + layout prep: transpose/cast to the DRAM layouts the
    kernel expects.  Returns in_maps for run_bass_kernel_spmd."""
    B, S, DT = text_vec.shape
    L, E = labels_vec.shape
    NL16 = L // 128 - 2 * NP8
    # W packed e-major into SBUF tile order: [128, E/128, DT/128, 128]
    wt = np.ascontiguousarray(
        W_proj.reshape(E // 128, 128, DT // 128, 128)
        .transpose(3, 0, 2, 1)).astype(F16)
    lt = np.ascontiguousarray(labels_vec.T).astype(F16)            # [E, L]
    laug = np.zeros((NL16 * 128, E + 2), dtype=F16)
    laug[:, :E] = labels_vec[:NL16 * 128].astype(F16)
    laug[:, E] = 1.0
    la8 = np.zeros((NP8 * 2 * 128, EPAD), dtype=F8)
    la8[:, :E] = labels_vec[NL16 * 128:].astype(F8)
    la8[:, E] = 1.0
    # constant correction: column sums of the EXACT fp8 label values (+count
    # for the rowsum column), pre-divided by 128 for the ones-stationary
    # matmul that broadcasts it over all s rows
    c8v = np.zeros(E + 2, dtype=np.float32)
    c8v[:E] = la8[:, :E].astype(np.float32).sum(axis=0)
    c8rep = np.tile(c8v[None, :], (128, 1)).astype(np.float32)
    b32 = np.ascontiguousarray(b_proj).astype(np.float32)
    SC = 512
    in_maps = []
    for b in range(B):
        # text^T packed chunk-major: [128, S/SC, DT/128, SC]
        ttb = np.ascontiguousarray(
            text_vec[b].T.reshape(DT // 128, 128, S // SC, SC)
            .transpose(1, 2, 0, 3)).astype(F16)
        in_maps.append({"tt": ttb, "wt": wt, "lt": lt, "laug": laug,
                        "la8": la8, "c8": c8rep, "bb": b32})
    return in_maps


def kernel(text_vec, labels_vec, W_proj, b_proj):
    from concourse.bass_utils import run_bass_kernel_spmd

    text_vec = np.asarray(text_vec)
    labels_vec = np.asarray(labels_vec)
    W_proj = np.asarray(W_proj)
    b_proj = np.asarray(b_proj)

    B, S, DT = text_vec.shape
    L, E = labels_vec.shape
    nc = _get_program((S, DT, E, L, 512))
    in_maps = prep_inputs(text_vec, labels_vec, W_proj, b_proj)

    trace = bool(int(os.environ.get("AMP_TRACE", "0")))
    res = run_bass_kernel_spmd(nc, in_maps, core_ids=list(range(B)), trace=trace)
    if trace and res.exec_time_ns is not None:
        print(f"HW exec time: {res.exec_time_ns} ns")
        if res.instructions_and_trace is not None:
            print(f"trace: {res.instructions_and_trace[1]}")
    out = np.stack([res.results[b]["out"] for b in range(B)], axis=0)
    return out.astype(np.float32)
